# revision 1
# baseline (speedup 1.0000x reference)
"""Trainium2 Bass kernel for nn_AttentionSelector (segment softmax attention).

Math shortcut: logits = segment_sum(w * repre) @ relation_mat.T + bias is
linear in repre, so with P = repre @ relation_mat.T ([N,53]) the whole
computation lives in 53-dim space:
    x_i   = P[i, labels[i]]          (rel logit per instance)
    e_i   = exp(x_i)                 (logits are ~N(0, 0.026^2): no max needed)
    out_b = (sum_{i in b} e_i P[i,:]) / (sum_{i in b} e_i) + bias

Device pipeline (per core, bags sharded 3125/core, rows padded to Rpad):
  A:  stream X^T in bf16 (the HBM roofline) as [128, 6*1024]-blocks
      (128-partition, 12KB contiguous lines -> ~340GB/s vs 135 for 2KB
      packets); 6 accumulating matmuls (D padded 690->768=6*128) per
      512-col half -> P^T in PSUM; scalar-engine copy to bf16 SBUF.
  T:  PE-transposes P^T -> row-major 128-row chunks (4 chunks per PSUM
      tile); per chunk: fused DVE multiply-reduce against a host-built
      one-hot label mask extracts x; scalar engine exp writes e directly
      into the P_aug e-column; pool engine scales P by e into P_aug and
      builds the one-hot slot matrix H0 from host-built seg ids.
  C:  per chunk one matmul att_un[s,0:54] = sum_i H0[i,s]*[eP | e][i,:]
      accumulated 9 chunks per PSUM bank; DVE copies groups to SBUF.
  Host compacts the <=2 (chunk,slot) partials per bag, divides by den,
  adds bias. All DMAs are large contiguous-per-partition transfers
  (no 4-byte scatter packets anywhere).
"""
import math
import os
import sys

for _p in ("/opt/trn_rl_repo", "/opt/trn_rl_repo/concourse", "/opt/pypackages"):
    if _p not in sys.path:
        sys.path.insert(0, _p)

import numpy as np
import ml_dtypes

BF16 = ml_dtypes.bfloat16
FP8 = ml_dtypes.float8_e4m3fn

N_TOTAL = 200000
NUM_BAGS = 25000
DIM = 690
DPAD = 768         # 6 * 128
KCH = 128
NK = 6
REL = 53
AUG = REL + 1      # 53 P-columns + e column
GRP = 9            # attention chunks accumulated per PSUM bank (9*54=486 f32)
NCORES = 8

LAST_RESULTS = None
_PROGRAM_CACHE = {}


def _build_program(Rpad, debug_out=False, stages="ATC", scalar_dma=True):
    from concourse import bacc, mybir
    import concourse.tile as tile
    from concourse.masks import make_identity

    f32 = mybir.dt.float32
    bf16 = mybir.dt.bfloat16
    fp8 = mybir.dt.float8e4
    Alu = mybir.AluOpType
    Act = mybir.ActivationFunctionType
    NJ = Rpad // 1024
    NCH = Rpad // 128
    T = 2 * NJ                      # number of 512-col halves

    nc = bacc.Bacc("TRN2", target_bir_lowering=False, debug=False,
                   enable_asserts=False)

    with tile.TileContext(nc) as tc:
        with tc.tile_pool(name="dram", bufs=1, space="DRAM") as dram, \
             tc.tile_pool(name="consts", bufs=1) as consts, \
             tc.tile_pool(name="xt", bufs=4) as xtp, \
             tc.tile_pool(name="junk", bufs=4) as junkp, \
             tc.tile_pool(name="pte", bufs=3) as ptep, \
             tc.tile_pool(name="erow", bufs=3) as erp, \
             tc.tile_pool(name="big", bufs=1) as bigp, \
             tc.tile_pool(name="pt_ps", bufs=2, space="PSUM") as ptps, \
             tc.tile_pool(name="tr_ps", bufs=2, space="PSUM") as trps, \
             tc.tile_pool(name="x_ps", bufs=2, space="PSUM") as xps, \
             tc.tile_pool(name="c_ps", bufs=2, space="PSUM") as cps:

            # merged per-block stream: [xt bf16 | h0 fp8 | ot fp8] with
            # uniform 14336-byte partition lines (one DMA per block)
            BCOLS = NK * 1024 + 512 + 1024      # in bf16 columns
            H0OFF = NK * 1024                   # h0: 512 bf16 cols = 1024 fp8
            OTOFF = NK * 1024 + 512             # ot: 1024 bf16 cols
            xt_d = dram.tile([NJ, 128, BCOLS], bf16, kind="ExternalInput",
                             name="xtb", uniquify=False)
            wm_d = dram.tile([128, NK, REL], bf16, kind="ExternalInput",
                             name="wmb", uniquify=False)
            att_d = dram.tile([128, NCH * AUG], bf16, kind="ExternalOutput",
                              name="attstage", uniquify=False)
            # constants
            wm_sb = consts.tile([128, NK, REL], bf16, name="wm_sb", tag="wm_sb")
            nc.sync.dma_start(wm_sb[:], wm_d[:])
            identb = consts.tile([128, 128], bf16, name="identb", tag="identb")
            make_identity(nc, identb[:])
            onesb = consts.tile([REL, AUG], bf16, name="onesb", tag="onesb")
            nc.vector.memset(onesb[:], 1.0)

            P_all = bigp.tile([128, NCH * AUG], bf16, name="P_all",
                              tag="P_all")
            attst = bigp.tile([128, NCH * AUG], bf16, name="attst",
                              tag="attst")
            if "C" not in stages:
                nc.vector.memset(attst[:], 0.0)
            # static P^T staging tiles with a permanent ones-row: transposing
            # [54,128] blocks yields [P | 1] slots, so den comes for free
            pt_sbs = []
            for i in range(4):
                t_ = consts.tile([AUG, 512], bf16, name=f"pt_sb{i}",
                                 tag=f"pt_sb{i}")
                nc.vector.memset(t_[:], 1.0)
                pt_sbs.append(t_)

            xt_tiles = {}
            pt_tiles = {}
            junk_tiles = {}
            pte_tiles = {}
            tr_tiles = {}
            cgrp = {"tile": None, "base": 0, "cnt": 0}

            out_state = {"done": 0}

            def flush_cgrp(final=False):
                t_, base, cnt = cgrp["tile"], cgrp["base"], cgrp["cnt"]
                if t_ is not None and cnt > 0:
                    nc.vector.tensor_copy(
                        attst[:, AUG * base:AUG * (base + cnt)],
                        t_[:, :AUG * cnt])
                    cgrp["tile"] = None
                    cgrp["cnt"] = 0
                # stream finished attst ranges out instead of one tail DMA
                hi = base + cnt if t_ is not None else out_state["done"]
                if final:
                    hi = NCH
                if hi - out_state["done"] >= 45 or \
                        (final and hi > out_state["done"]):
                    lo = out_state["done"]
                    nc.scalar.dma_start(att_d[:, AUG * lo:AUG * hi],
                                      attst[:, AUG * lo:AUG * hi])
                    out_state["done"] = hi

            for t in range(T + 5):
                # ---- stage A: matmuls for half t ----
                if t < T:
                    j, h = divmod(t, 2)
                    if h == 0:
                        xt = xtp.tile([128, BCOLS], bf16, name="xt",
                                      tag="xt")
                        nc.sync.dma_start(xt[:], xt_d[j])
                        xt_tiles[j] = xt
                    xt = xt_tiles[j]
                    pt_ps = ptps.tile([REL, 512], f32, space="PSUM",
                                      name="pt_ps", tag="pt_ps")
                    for k in range(NK):
                        nc.tensor.matmul(
                            pt_ps[:], wm_sb[:, k, :],
                            xt[:, 1024 * k + 512 * h:1024 * k + 512 * (h + 1)],
                            start=(k == 0), stop=(k == NK - 1))
                    pt_sb = pt_sbs[t % 4]
                    nc.scalar.activation(pt_sb[:REL, :], pt_ps[:], Act.Copy)
                    pt_tiles[t] = pt_sb
                    junk = junkp.tile([REL, 512], bf16, name="junk",
                                      tag="junk")
                    nc.vector.tensor_tensor(
                        out=junk[:], in0=pt_sb[:REL, :],
                        in1=xt[:REL, OTOFF + 512 * h:OTOFF + 512 * (h + 1)],
                        op=Alu.mult)
                    junk_tiles[t] = junk

                # ---- stage X: x^T, e^T, column-scale for half t-2 ----
                u1 = t - 2
                if "T" in stages and 0 <= u1 < T:
                    junk = junk_tiles.pop(u1)
                    xT_ps = xps.tile([AUG, 512], f32, space="PSUM",
                                     name="xT", tag="xT")
                    nc.tensor.matmul(xT_ps[:], onesb[:], junk[:],
                                     start=True, stop=True)
                    e_bc = erp.tile([AUG, 512], bf16, name="erow", tag="erow")
                    nc.scalar.activation(e_bc[:], xT_ps[:], Act.Exp)
                    pt_e = ptep.tile([AUG, 512], bf16, name="pte", tag="pte")
                    nc.vector.tensor_tensor(
                        out=pt_e[:], in0=pt_tiles.pop(u1),
                        in1=e_bc[:], op=Alu.mult)
                    pte_tiles[u1] = pt_e

                # ---- stage T: transposes + P_all copy for half t-3 ----
                u = t - 3
                if "T" in stages and 0 <= u < T:
                    pt_e = pte_tiles.pop(u)
                    c0 = 4 * u
                    tr4 = trps.tile([128, 4 * AUG], bf16, space="PSUM",
                                    name="tr4", tag="tr4")
                    for q in range(4):
                        nc.tensor.transpose(
                            tr4[:, AUG * q:AUG * (q + 1)],
                            pt_e[:, 128 * q:128 * (q + 1)],
                            identb[:AUG, :AUG])
                    tr_tiles[u] = tr4
                    nc.scalar.activation(
                        P_all[:, AUG * c0:AUG * (c0 + 4)], tr4[:], Act.Copy)

                # ---- stage C: attention matmuls for half t-4 ----
                v = t - 4
                if "C" in stages and 0 <= v < T:
                    tr_tiles.pop(v, None)
                    for q in range(4):
                        c = 4 * v + q
                        if cgrp["tile"] is None:
                            cgrp["tile"] = cps.tile(
                                [128, GRP * AUG], f32, space="PSUM",
                                name="cacc", tag="cacc")
                            cgrp["base"] = c
                        off = AUG * (c - cgrp["base"])
                        cj, cq = divmod(c, 8)
                        h0v = xt_tiles[cj][:, H0OFF + 64 * cq:
                                           H0OFF + 64 * (cq + 1)].bitcast(fp8)
                        nc.tensor.matmul(
                            cgrp["tile"][:, off:off + AUG], h0v,
                            P_all[:, AUG * c:AUG * (c + 1)],
                            start=True, stop=True)
                        cgrp["cnt"] += 1
                        if cgrp["cnt"] == GRP:
                            flush_cgrp()
            flush_cgrp(final=True)

    nc.compile()
    return nc


def _prep(repre, relation_mat, bias, scope, labels, ncores):
    repre = np.asarray(repre, dtype=np.float32)
    relmat = np.asarray(relation_mat, dtype=np.float32)
    bias_np = np.asarray(bias, dtype=np.float32)
    scope = np.asarray(scope).astype(np.int64)
    labels_np = np.asarray(labels).astype(np.int64)
    n, d = repre.shape
    nbags = scope.shape[0]
    assert d == DIM and nbags % ncores == 0
    bpc = nbags // ncores
    starts, ends = scope[:, 0], scope[:, 1]
    lens = ends - starts
    core_r0 = starts[np.arange(ncores) * bpc]
    core_r1 = ends[np.arange(ncores) * bpc + bpc - 1]
    rows = core_r1 - core_r0
    Rpad = int(1024 * math.ceil(int(rows.max()) / 1024))
    NCH = Rpad // 128
    NJ = Rpad // 1024
    assert int(lens.max()) <= 128, "bag too large for this kernel layout"

    wmb = np.zeros((128, NK, REL), np.float32)
    for k in range(NK):
        lo, hi = k * KCH, min((k + 1) * KCH, DIM)
        wmb[:hi - lo, k, :] = relmat[:, lo:hi].T
    wmb = wmb.astype(BF16)

    in_maps, metas = [], []
    for c in range(ncores):
        r0, r1 = int(core_r0[c]), int(core_r1[c])
        rc = r1 - r0
        Xp = np.zeros((Rpad, DPAD), np.float32)
        Xp[:rc, :DIM] = repre[r0:r1]
        xt_part = np.ascontiguousarray(
            Xp.reshape(NJ, 1024, NK, 128).transpose(0, 3, 2, 1)
            .reshape(NJ, 128, NK * 1024)).astype(BF16)

        lab = labels_np[r0:r1]
        O = np.zeros((Rpad, REL), np.float32)
        O[np.arange(rc), lab] = 1.0
        OTfull = np.zeros((128, Rpad), np.float32)
        OTfull[:REL] = O.T
        ot_part = np.ascontiguousarray(
            OTfull.reshape(128, NJ, 1024).transpose(1, 0, 2)).astype(BF16)

        blens = lens[c * bpc:(c + 1) * bpc]
        segl = np.repeat(np.arange(bpc, dtype=np.int64), blens)
        seg_pad = np.concatenate(
            [segl, bpc + np.arange(Rpad - rc, dtype=np.int64)])
        chunk_first = seg_pad[(np.arange(Rpad) // 128) * 128]
        seg_local = seg_pad - chunk_first
        assert seg_local.max() <= 127
        H = (seg_local.reshape(NCH, 128)[:, :, None]
             == np.arange(128)[None, None, :])
        h0_part = np.ascontiguousarray(
            H.transpose(1, 0, 2).reshape(128, NCH * 128)
            .reshape(128, NJ, 1024).transpose(1, 0, 2)).astype(FP8)

        xtb = np.concatenate([
            xt_part.view(np.uint8),
            h0_part.view(np.uint8).reshape(NJ, 128, 1024),
            ot_part.view(np.uint8).reshape(NJ, 128, 2048),
        ], axis=2).view(BF16)

        in_maps.append({"xtb": xtb, "wmb": wmb})

        ls = starts[c * bpc:(c + 1) * bpc] - r0
        le = ends[c * bpc:(c + 1) * bpc] - r0
        k0 = ls // 128
        k1 = (le - 1) // 128
        bidx = np.arange(bpc, dtype=np.int64)
        slot0 = bidx - chunk_first[k0 * 128]
        slot1 = bidx - chunk_first[k1 * 128]
        assert slot0.min() >= 0 and slot0.max() <= 127
        assert slot1.min() >= 0 and slot1.max() <= 127
        metas.append((k0, slot0, k1, slot1))
    return in_maps, metas, bias_np, Rpad, bpc


def _compact(results, metas, bias_np, bpc, Rpad):
    NCH = Rpad // 128
    out = np.empty((len(results) * bpc, REL), np.float32)
    for c, res in enumerate(results):
        stage = np.asarray(res["attstage"]).astype(np.float32) \
            .reshape(128, NCH, AUG)
        k0, slot0, k1, slot1 = metas[c]
        acc = stage[slot0, k0, :].copy()
        two = k1 > k0
        acc[two] += stage[slot1[two], k1[two], :]
        out[c * bpc:(c + 1) * bpc] = acc[:, :REL] / acc[:, REL:AUG]
    out += bias_np[None, :]
    return out


def kernel(repre, relation_mat, bias, scope, labels):
    global LAST_RESULTS
    from concourse.bass_utils import run_bass_kernel_spmd

    in_maps, metas, bias_np, Rpad, bpc = _prep(
        repre, relation_mat, bias, scope, labels, NCORES)
    if Rpad not in _PROGRAM_CACHE:
        _PROGRAM_CACHE[Rpad] = _build_program(Rpad)
    nc = _PROGRAM_CACHE[Rpad]
    res = run_bass_kernel_spmd(nc, in_maps, core_ids=list(range(NCORES)),
                               trace=bool(os.environ.get("BASS_TRACE")))
    LAST_RESULTS = res
    return _compact(res.results, metas, bias_np, bpc, Rpad)



# revision 7
# speedup vs baseline: 1.4250x; 1.4250x over previous
"""Trainium2 Bass kernel for nn_AttentionSelector (segment softmax attention).

Math shortcut: logits = segment_sum(w * repre) @ relation_mat.T + bias is
linear in repre, so with P = repre @ relation_mat.T ([N,53]) the whole
computation lives in 53-dim space:
    x_i   = P[i, labels[i]]          (rel logit per instance)
    e_i   = exp(x_i)                 (logits are ~N(0, 0.026^2): no max needed)
    out_b = (sum_{i in b} e_i P[i,:]) / (sum_{i in b} e_i) + bias

Device pipeline (per core, bags sharded 3125/core, rows padded to Rpad):
  The X^T stream is the HBM roofline; everything else hides under it.
  Layout per 1024-row block: 5 full 128-d chunks ([128,1024] each) plus
  the 50-dim tail dual-packed (half 0 at partitions 0-49, half 1 at
  64-113) -> 5632 bf16 cols = 11264 B/partition, 98% of the line is
  real data (no 690->768 zero padding, no one-hot streams).
  Per 512-instance half t (even halves at partition base 0, odd at 64 -
  PE tile_position col groups make the shifted pipeline free, and the
  0/64 interleave gives a full-width [128, *] output stage):
    A:  5 accumulating matmuls + 1 tail matmul (K=50 at base 64h)
        -> P^T in PSUM [53,512]; ACT copies to bf16 pt tile whose
        row b+53 is a permanent ones-row.
    X:  K=1 matmul broadcasts labels row across 53 partitions; one fused
        DVE scalar_tensor_tensor computes junk = (lb == iota) * P^T;
        ones-matmul contracts partitions -> x broadcast to 54 rows in
        PSUM; ACT exp -> e rows; DVE multiplies [P^T; 1] by e writing
        [e P | e] straight into the [128, Rpad/2] output staging tile.
  Input streams as 5.8 MB 4-block DMAs (sync engine), output flushes as
  [128, 2048] slices (scalar engine queue). Host rebuilds [N, 54],
  segment-sums contiguous bags via cumsum-diff, divides, adds bias.
"""
import math
import os
import sys

for _p in ("/opt/trn_rl_repo", "/opt/trn_rl_repo/concourse", "/opt/pypackages"):
    if _p not in sys.path:
        sys.path.insert(0, _p)

import numpy as np
import ml_dtypes

BF16 = ml_dtypes.bfloat16

N_TOTAL = 200000
NUM_BAGS = 25000
DIM = 690
KCH = 128
NKF = 5            # full 128-d chunks
DTAIL = DIM - NKF * KCH          # 50
REL = 53
AUG = REL + 1      # 53 P-columns + e column
AUGW = 64          # widened row count so outst rows 54-63/118-127 are
                   # written too (host ignores them; keeps DMA full-width)
BCOL = NKF * 1024 + 512          # 5632 bf16 cols per 1024-row block
NCORES = 8
GROUP_BLOCKS = 4                 # steady-state blocks per input DMA
FLUSH_BLOCKS = 4                 # output flush granularity (blocks)


def _in_groups(NJ):
    groups = [1]
    left = NJ - 1
    while left > 0:
        g = min(GROUP_BLOCKS, left)
        groups.append(g)
        left -= g
    return groups

LAST_RESULTS = None
_PROGRAM_CACHE = {}


def _build_program(Rpad):
    from concourse import bacc, mybir
    import concourse.tile as tile

    f32 = mybir.dt.float32
    bf16 = mybir.dt.bfloat16
    Alu = mybir.AluOpType
    Act = mybir.ActivationFunctionType
    NJ = Rpad // 1024
    T = 2 * NJ
    IN_GROUPS = _in_groups(NJ)

    nc = bacc.Bacc("TRN2", target_bir_lowering=False, debug=False,
                   enable_asserts=False)

    with tile.TileContext(nc) as tc:
        with tc.tile_pool(name="dram", bufs=1, space="DRAM") as dram, \
             tc.tile_pool(name="consts", bufs=1) as consts, \
             tc.tile_pool(name="xt", bufs=2) as xtp, \
             tc.tile_pool(name="junk", bufs=3) as junkp, \
             tc.tile_pool(name="erow", bufs=3) as erp, \
             tc.tile_pool(name="big", bufs=1) as bigp, \
             tc.tile_pool(name="pt_ps", bufs=2, space="PSUM") as ptps, \
             tc.tile_pool(name="lb_ps", bufs=2, space="PSUM") as lbps, \
             tc.tile_pool(name="x_ps", bufs=2, space="PSUM") as xps:

            xt_d = dram.tile([128, NJ * BCOL], bf16, kind="ExternalInput",
                             name="xtb", uniquify=False)
            wm_d = dram.tile([128, NKF * REL + REL + 1], bf16,
                             kind="ExternalInput", name="wmb",
                             uniquify=False)
            lab_d = dram.tile([1, Rpad], bf16, kind="ExternalInput",
                              name="labb", uniquify=False)
            out_d = dram.tile([128, Rpad // 2], bf16, kind="ExternalOutput",
                              name="outstage", uniquify=False)

            # constants: [wm 5*53 | wm_tail 53 | iota 1]
            wm_sb = consts.tile([128, NKF * REL + REL + 1], bf16,
                                name="wm_sb", tag="wm_sb")
            nc.sync.dma_start(wm_sb[:], wm_d[:])
            lab_sb = consts.tile([1, Rpad], bf16, name="lab_sb",
                                 tag="lab_sb")
            nc.sync.dma_start(lab_sb[:], lab_d[:])
            WT = NKF * REL                 # wm_tail col offset
            IOTA = WT + REL                # iota col offset

            onesb = consts.tile([128, AUGW], bf16, name="onesb", tag="onesb")
            nc.vector.memset(onesb[:], 1.0)
            ones1 = consts.tile([1, REL], bf16, name="ones1", tag="ones1")
            nc.vector.memset(ones1[:], 1.0)
            # P^T staging tiles with permanent ones-rows at 53 and 117
            pt_sbs = []
            for i in range(4):
                t_ = consts.tile([128, 512], bf16, name=f"pt_sb{i}",
                                 tag=f"pt_sb{i}")
                nc.vector.memset(t_[:], 1.0)
                pt_sbs.append(t_)

            outst = bigp.tile([128, Rpad // 2], bf16, name="outst",
                              tag="outst")

            xt_tiles = {}
            pt_ps_t = {}
            lb_ps_t = {}
            group_of_block = {}
            g0 = 0
            group_bounds = []
            for g in IN_GROUPS:
                group_bounds.append((g0, g0 + g))
                for j in range(g0, g0 + g):
                    group_of_block[j] = len(group_bounds) - 1
                g0 += g

            out_state = {"done": 0}

            def maybe_flush(jdone, final=False):
                # block jdone fully written; flush completed col ranges
                hi = (jdone + 1) * 512
                if final:
                    hi = (Rpad // 2)
                if hi - out_state["done"] >= FLUSH_BLOCKS * 512 or \
                        (final and hi > out_state["done"]):
                    lo = out_state["done"]
                    nc.scalar.dma_start(out_d[:, lo:hi], outst[:, lo:hi])
                    out_state["done"] = hi

            for t in range(T + 1):
                # ---- stage X part 1 for half t-1: label broadcast ----
                u = t - 1
                if 0 <= u < T:
                    ju, hu = divmod(u, 2)
                    bu = 64 * hu
                    lb_ps = lbps.tile([128, 512], f32, space="PSUM",
                                      name="lb_ps", tag="lb_ps")
                    nc.tensor.matmul(
                        lb_ps[bu:bu + REL, :], ones1[:, :],
                        lab_sb[:, 512 * u:512 * (u + 1)],
                        start=True, stop=True)
                    lb_ps_t[u] = lb_ps
                    pt_sb = pt_sbs[u % 4]
                    nc.scalar.activation(
                        pt_sb[bu:bu + REL, :],
                        pt_ps_t[u][bu:bu + REL, :], Act.Copy)
                    junk = junkp.tile([128, 512], bf16, name="junk",
                                      tag="junk")
                    # junk = (lb == iota) * P^T  in one DVE op
                    nc.vector.scalar_tensor_tensor(
                        out=junk[bu:bu + REL, :],
                        in0=lb_ps[bu:bu + REL, :],
                        scalar=wm_sb[bu:bu + REL, IOTA:IOTA + 1],
                        in1=pt_sb[bu:bu + REL, :],
                        op0=Alu.is_equal, op1=Alu.mult)

                # ---- stage A: matmuls for half t ----
                if t < T:
                    j, h = divmod(t, 2)
                    b = 64 * h
                    gi = group_of_block[j]
                    if j == group_bounds[gi][0] and h == 0:
                        glo, ghi = group_bounds[gi]
                        xt = xtp.tile([128, GROUP_BLOCKS * BCOL], bf16,
                                      name="xt", tag="xt")
                        nc.sync.dma_start(
                            xt[:, :(ghi - glo) * BCOL],
                            xt_d[:, glo * BCOL:ghi * BCOL])
                        for jj in range(glo, ghi):
                            xt_tiles[jj] = (xt, (jj - glo) * BCOL)
                    xt, off = xt_tiles[j]
                    pt_ps = ptps.tile([128, 512], f32, space="PSUM",
                                      name="pt_ps", tag="pt_ps")
                    for k in range(NKF):
                        nc.tensor.matmul(
                            pt_ps[b:b + REL, :],
                            wm_sb[:, REL * k:REL * (k + 1)],
                            xt[:, off + 1024 * k + 512 * h:
                               off + 1024 * k + 512 * (h + 1)],
                            start=(k == 0), stop=False)
                    nc.tensor.matmul(
                        pt_ps[b:b + REL, :],
                        wm_sb[b:b + DTAIL, WT:WT + REL],
                        xt[b:b + DTAIL, off + NKF * 1024:
                           off + NKF * 1024 + 512],
                        start=False, stop=True)
                    pt_ps_t[t] = pt_ps

                # ---- stage X part 2 for half t-1 ----
                if 0 <= u < T:
                    ju, hu = divmod(u, 2)
                    bu = 64 * hu
                    junk = junk  # noqa -- from part 1 above
                    xT_ps = xps.tile([128, 512], f32, space="PSUM",
                                     name="xT", tag="xT")
                    nc.tensor.matmul(
                        xT_ps[bu:bu + AUGW, :], onesb[bu:bu + REL, :],
                        junk[bu:bu + REL, :], start=True, stop=True)
                    e_bc = erp.tile([128, 512], bf16, name="erow",
                                    tag="erow")
                    nc.scalar.activation(
                        e_bc[bu:bu + AUGW, :], xT_ps[bu:bu + AUGW, :],
                        Act.Exp)
                    pt_sb = pt_sbs[u % 4]
                    nc.vector.tensor_tensor(
                        out=outst[bu:bu + AUGW, 512 * ju:512 * (ju + 1)],
                        in0=pt_sb[bu:bu + AUGW, :],
                        in1=e_bc[bu:bu + AUGW, :], op=Alu.mult)
                    pt_ps_t.pop(u)
                    lb_ps_t.pop(u)
                    if hu == 1:
                        maybe_flush(ju)
            maybe_flush(NJ - 1, final=True)

    nc.compile()
    return nc


def _prep(repre, relation_mat, bias, scope, labels, ncores):
    repre = np.asarray(repre, dtype=np.float32)
    relmat = np.asarray(relation_mat, dtype=np.float32)
    bias_np = np.asarray(bias, dtype=np.float32)
    scope = np.asarray(scope).astype(np.int64)
    labels_np = np.asarray(labels).astype(np.int64)
    n, d = repre.shape
    nbags = scope.shape[0]
    assert d == DIM and nbags % ncores == 0
    bpc = nbags // ncores
    starts, ends = scope[:, 0], scope[:, 1]
    core_r0 = starts[np.arange(ncores) * bpc]
    core_r1 = ends[np.arange(ncores) * bpc + bpc - 1]
    rows = core_r1 - core_r0
    Rpad = int(1024 * math.ceil(int(rows.max()) / 1024))
    NJ = Rpad // 1024

    # constants: [wm 5*53 | wm_tail 53 | iota 1]
    wmb = np.zeros((128, NKF * REL + REL + 1), np.float32)
    for k in range(NKF):
        wmb[:, REL * k:REL * (k + 1)] = relmat[:, KCH * k:KCH * (k + 1)].T
    wmb[0:DTAIL, NKF * REL:NKF * REL + REL] = relmat[:, NKF * KCH:].T
    wmb[64:64 + DTAIL, NKF * REL:NKF * REL + REL] = relmat[:, NKF * KCH:].T
    iota = np.zeros(128, np.float32)
    iota[:REL] = np.arange(REL)
    iota[64:64 + REL] = np.arange(REL)
    wmb[:, NKF * REL + REL] = iota
    wmb = wmb.astype(BF16)

    in_maps, metas = [], []
    for c in range(ncores):
        r0, r1 = int(core_r0[c]), int(core_r1[c])
        rc = r1 - r0
        Xc = np.zeros((Rpad, DIM), np.float32)
        Xc[:rc] = repre[r0:r1]
        M = Xc.reshape(NJ, 2, 512, DIM).transpose(3, 0, 1, 2)  # [690,NJ,2,512]
        main = (M[:NKF * KCH].reshape(NKF, KCH, NJ, 2, 512)
                .transpose(1, 2, 0, 3, 4).reshape(KCH, NJ, NKF * 1024))
        tail = np.zeros((KCH, NJ, 512), np.float32)
        tail[0:DTAIL] = M[NKF * KCH:, :, 0, :]
        tail[64:64 + DTAIL] = M[NKF * KCH:, :, 1, :]
        xtb = np.ascontiguousarray(
            np.concatenate([main, tail], axis=2).reshape(128, NJ * BCOL)
        ).astype(BF16)

        lab = np.zeros((1, Rpad), np.float32)
        lab[0, :rc] = labels_np[r0:r1]
        labb = lab.astype(BF16)

        in_maps.append({"xtb": xtb, "wmb": wmb, "labb": labb})
        metas.append((r0, starts[c * bpc:(c + 1) * bpc] - r0,
                      ends[c * bpc:(c + 1) * bpc] - r0, rc))
    return in_maps, metas, bias_np, Rpad, bpc


def _finish(results, metas, bias_np, bpc, Rpad):
    NJ = Rpad // 1024
    out = np.empty((len(results) * bpc, REL), np.float32)
    for c, res in enumerate(results):
        arr = np.asarray(res["outstage"]).astype(np.float32)
        A = arr.reshape(128, NJ, 512)
        pte = np.empty((NJ, 2, 512, AUG), np.float32)   # [block, half, i, r]
        pte[:, 0] = A[0:AUG].transpose(1, 2, 0)
        pte[:, 1] = A[64:64 + AUG].transpose(1, 2, 0)
        pte = pte.reshape(Rpad, AUG)
        _, ls, le, rc = metas[c]
        cs = np.empty((Rpad + 1, AUG), np.float64)
        cs[0] = 0.0
        np.cumsum(pte, axis=0, dtype=np.float64, out=cs[1:])
        sums = cs[le] - cs[ls]
        out[c * bpc:(c + 1) * bpc] = (
            sums[:, :REL] / sums[:, REL:AUG]).astype(np.float32)
    out += bias_np[None, :]
    return out


def kernel(repre, relation_mat, bias, scope, labels):
    global LAST_RESULTS
    from concourse.bass_utils import run_bass_kernel_spmd

    in_maps, metas, bias_np, Rpad, bpc = _prep(
        repre, relation_mat, bias, scope, labels, NCORES)
    if Rpad not in _PROGRAM_CACHE:
        _PROGRAM_CACHE[Rpad] = _build_program(Rpad)
    nc = _PROGRAM_CACHE[Rpad]
    res = run_bass_kernel_spmd(nc, in_maps, core_ids=list(range(NCORES)),
                               trace=bool(os.environ.get("BASS_TRACE")))
    LAST_RESULTS = res
    return _finish(res.results, metas, bias_np, bpc, Rpad)


# revision 9
# speedup vs baseline: 1.4937x; 1.0482x over previous
"""Trainium2 Bass kernel for nn_AttentionSelector (segment softmax attention).

Math shortcut: logits = segment_sum(w * repre) @ relation_mat.T + bias is
linear in repre, so with P = repre @ relation_mat.T ([N,53]) the whole
computation lives in 53-dim space:
    x_i   = P[i, labels[i]]          (rel logit per instance)
    e_i   = exp(x_i)                 (logits are ~N(0, 0.026^2): no max needed)
    out_b = (sum_{i in b} e_i P[i,:]) / (sum_{i in b} e_i) + bias

Device pipeline (per core, bags sharded 3125/core, rows padded to Rpad):
  The X^T stream is the HBM roofline; everything else hides under it.
  Layout per 1024-row block: 5 full 128-d chunks ([128,1024] each) plus
  the 50-dim tail dual-packed (half 0 at partitions 0-49, half 1 at
  64-113) -> 5632 bf16 cols = 11264 B/partition, 98% of the line is
  real data (no 690->768 zero padding, no one-hot streams).
  Per 512-instance half t (even halves at partition base 0, odd at 64 -
  PE tile_position col groups make the shifted pipeline free, and the
  0/64 interleave gives a full-width [128, *] output stage):
    A:  5 accumulating matmuls + 1 tail matmul (K=50 at base 64h)
        -> P^T in PSUM [53,512]; ACT copies to bf16 pt tile whose
        row b+53 is a permanent ones-row.
    X:  K=1 matmul broadcasts labels row across 53 partitions; one fused
        DVE scalar_tensor_tensor computes junk = (lb == iota) * P^T;
        ones-matmul contracts partitions -> x broadcast to 54 rows in
        PSUM; ACT exp -> e rows; DVE multiplies [P^T; 1] by e writing
        [e P | e] straight into the [128, Rpad/2] output staging tile.
  Input streams as 5.8 MB 4-block DMAs (sync engine), output flushes as
  [128, 2048] slices (scalar engine queue). Host rebuilds [N, 54],
  segment-sums contiguous bags via cumsum-diff, divides, adds bias.
"""
import math
import os
import sys

for _p in ("/opt/trn_rl_repo", "/opt/trn_rl_repo/concourse", "/opt/pypackages"):
    if _p not in sys.path:
        sys.path.insert(0, _p)

import numpy as np
import ml_dtypes

BF16 = ml_dtypes.bfloat16

N_TOTAL = 200000
NUM_BAGS = 25000
DIM = 690
KCH = 128
NKF = 5            # full 128-d chunks
DTAIL = DIM - NKF * KCH          # 50
REL = 53
AUG = REL + 1      # 53 P-columns + e column
AUGW = 64          # widened row count so outst rows 54-63/118-127 are
                   # written too (host ignores them; keeps DMA full-width)
BCOL = NKF * 1024 + 512          # 5632 bf16 cols per 1024-row block
NCORES = 8
GROUP_BLOCKS = 4                 # steady-state blocks per input DMA
FLUSH_BLOCKS = 4                 # output flush granularity (blocks)


def _in_groups(NJ):
    groups = [1]
    left = NJ - 1
    while left > 0:
        g = min(GROUP_BLOCKS, left)
        groups.append(g)
        left -= g
    return groups

LAST_RESULTS = None
_PROGRAM_CACHE = {}


def _build_program(Rpad):
    from concourse import bacc, mybir
    import concourse.tile as tile

    f32 = mybir.dt.float32
    bf16 = mybir.dt.bfloat16
    Alu = mybir.AluOpType
    Act = mybir.ActivationFunctionType
    NJ = Rpad // 1024
    T = 2 * NJ
    IN_GROUPS = _in_groups(NJ)

    nc = bacc.Bacc("TRN2", target_bir_lowering=False, debug=False,
                   enable_asserts=False)

    with tile.TileContext(nc) as tc:
        with tc.tile_pool(name="dram", bufs=1, space="DRAM") as dram, \
             tc.tile_pool(name="consts", bufs=1) as consts, \
             tc.tile_pool(name="xt", bufs=2) as xtp, \
             tc.tile_pool(name="junk", bufs=3) as junkp, \
             tc.tile_pool(name="erow", bufs=3) as erp, \
             tc.tile_pool(name="big", bufs=1) as bigp, \
             tc.tile_pool(name="pt_ps", bufs=2, space="PSUM") as ptps, \
             tc.tile_pool(name="lb_ps", bufs=2, space="PSUM") as lbps, \
             tc.tile_pool(name="x_ps", bufs=2, space="PSUM") as xps:

            xt_d = dram.tile([128, NJ * BCOL], bf16, kind="ExternalInput",
                             name="xtb", uniquify=False)
            wm_d = dram.tile([128, NKF * REL + REL + 1], bf16,
                             kind="ExternalInput", name="wmb",
                             uniquify=False)
            lab_d = dram.tile([1, Rpad], bf16, kind="ExternalInput",
                              name="labb", uniquify=False)
            out_d = dram.tile([128, Rpad // 2], bf16, kind="ExternalOutput",
                              name="outstage", uniquify=False)

            # constants: [wm 5*53 | wm_tail 53 | iota 1]
            # (scalar queue, so the first xt group DMA leads the sync queue)
            wm_sb = consts.tile([128, NKF * REL + REL + 1], bf16,
                                name="wm_sb", tag="wm_sb")
            nc.scalar.dma_start(wm_sb[:], wm_d[:])
            lab_sb = consts.tile([1, Rpad], bf16, name="lab_sb",
                                 tag="lab_sb")
            nc.scalar.dma_start(lab_sb[:], lab_d[:])
            WT = NKF * REL                 # wm_tail col offset
            IOTA = WT + REL                # iota col offset

            onesb = consts.tile([128, AUGW], bf16, name="onesb", tag="onesb")
            nc.vector.memset(onesb[:], 1.0)
            ones1 = consts.tile([1, REL], bf16, name="ones1", tag="ones1")
            nc.vector.memset(ones1[:], 1.0)
            # P^T staging tiles with permanent ones-rows at 53 and 117
            pt_sbs = []
            for i in range(4):
                t_ = consts.tile([128, 512], bf16, name=f"pt_sb{i}",
                                 tag=f"pt_sb{i}")
                nc.vector.memset(t_[:], 1.0)
                pt_sbs.append(t_)

            outst = bigp.tile([128, Rpad // 2], bf16, name="outst",
                              tag="outst")

            xt_tiles = {}
            pt_ps_t = {}
            lb_ps_t = {}
            group_of_block = {}
            g0 = 0
            group_bounds = []
            for g in IN_GROUPS:
                group_bounds.append((g0, g0 + g))
                for j in range(g0, g0 + g):
                    group_of_block[j] = len(group_bounds) - 1
                g0 += g

            out_state = {"done": 0}

            def maybe_flush(jdone, final=False):
                # block jdone fully written; flush completed col ranges
                hi = (jdone + 1) * 512
                if final:
                    hi = (Rpad // 2)
                if hi - out_state["done"] >= FLUSH_BLOCKS * 512 or \
                        (final and hi > out_state["done"]):
                    lo = out_state["done"]
                    nc.scalar.dma_start(out_d[:, lo:hi], outst[:, lo:hi])
                    out_state["done"] = hi

            # pair p = both 512-halves of block p; even half at partition
            # base 0, odd at 64.  The two halves' matmuls are issued
            # interleaved so the PE runs them concurrently in disjoint
            # column groups (tile_position cols 0/64).
            for p in range(NJ + 1):
                # ---- stage X part 1 for pair p-1 ----
                u = p - 1
                if 0 <= u < NJ:
                    lb_ps = lbps.tile([128, 512], f32, space="PSUM",
                                      name="lb_ps", tag="lb_ps")
                    for hu in (0, 1):
                        bu = 64 * hu
                        nc.tensor.matmul(
                            lb_ps[bu:bu + REL, :], ones1[:, :],
                            lab_sb[:, 1024 * u + 512 * hu:
                                   1024 * u + 512 * (hu + 1)],
                            start=True, stop=True)
                    pt_sb = pt_sbs[u % 4]
                    junk = junkp.tile([128, 512], bf16, name="junk",
                                      tag="junk")
                    for hu in (0, 1):
                        bu = 64 * hu
                        nc.scalar.activation(
                            pt_sb[bu:bu + REL, :],
                            pt_ps_t[u][bu:bu + REL, :], Act.Copy)
                        # junk = (lb == iota) * P^T  in one DVE op
                        nc.vector.scalar_tensor_tensor(
                            out=junk[bu:bu + REL, :],
                            in0=lb_ps[bu:bu + REL, :],
                            scalar=wm_sb[bu:bu + REL, IOTA:IOTA + 1],
                            in1=pt_sb[bu:bu + REL, :],
                            op0=Alu.is_equal, op1=Alu.mult)

                # ---- stage A: interleaved matmuls for pair p ----
                if p < NJ:
                    j = p
                    gi = group_of_block[j]
                    if j == group_bounds[gi][0]:
                        glo, ghi = group_bounds[gi]
                        xt = xtp.tile([128, GROUP_BLOCKS * BCOL], bf16,
                                      name="xt", tag="xt")
                        nc.sync.dma_start(
                            xt[:, :(ghi - glo) * BCOL],
                            xt_d[:, glo * BCOL:ghi * BCOL])
                        for jj in range(glo, ghi):
                            xt_tiles[jj] = (xt, (jj - glo) * BCOL)
                    xt, off = xt_tiles[j]
                    pt_ps = ptps.tile([128, 512], f32, space="PSUM",
                                      name="pt_ps", tag="pt_ps")
                    for k in range(NKF):
                        for h in (0, 1):
                            b = 64 * h
                            nc.tensor.matmul(
                                pt_ps[b:b + REL, :],
                                wm_sb[:, REL * k:REL * (k + 1)],
                                xt[:, off + 1024 * k + 512 * h:
                                   off + 1024 * k + 512 * (h + 1)],
                                start=(k == 0), stop=False,
                                skip_group_check=True)
                    for h in (0, 1):
                        b = 64 * h
                        nc.tensor.matmul(
                            pt_ps[b:b + REL, :],
                            wm_sb[b:b + DTAIL, WT:WT + REL],
                            xt[b:b + DTAIL, off + NKF * 1024:
                               off + NKF * 1024 + 512],
                            start=False, stop=True,
                            skip_group_check=True)
                    pt_ps_t[p] = pt_ps

                # ---- stage X part 2 for pair p-1 ----
                if 0 <= u < NJ:
                    xT_ps = xps.tile([128, 512], f32, space="PSUM",
                                     name="xT", tag="xT")
                    for hu in (0, 1):
                        bu = 64 * hu
                        nc.tensor.matmul(
                            xT_ps[bu:bu + AUGW, :], onesb[bu:bu + REL, :],
                            junk[bu:bu + REL, :], start=True, stop=True)
                    e_bc = erp.tile([128, 512], bf16, name="erow",
                                    tag="erow")
                    pt_sb = pt_sbs[u % 4]
                    for hu in (0, 1):
                        bu = 64 * hu
                        nc.scalar.activation(
                            e_bc[bu:bu + AUGW, :], xT_ps[bu:bu + AUGW, :],
                            Act.Exp)
                        nc.vector.tensor_tensor(
                            out=outst[bu:bu + AUGW,
                                      512 * u:512 * (u + 1)],
                            in0=pt_sb[bu:bu + AUGW, :],
                            in1=e_bc[bu:bu + AUGW, :], op=Alu.mult)
                    pt_ps_t.pop(u)
                    maybe_flush(u)
            maybe_flush(NJ - 1, final=True)

    nc.compile()
    return nc


def _prep(repre, relation_mat, bias, scope, labels, ncores):
    repre = np.asarray(repre, dtype=np.float32)
    relmat = np.asarray(relation_mat, dtype=np.float32)
    bias_np = np.asarray(bias, dtype=np.float32)
    scope = np.asarray(scope).astype(np.int64)
    labels_np = np.asarray(labels).astype(np.int64)
    n, d = repre.shape
    nbags = scope.shape[0]
    assert d == DIM and nbags % ncores == 0
    bpc = nbags // ncores
    starts, ends = scope[:, 0], scope[:, 1]
    core_r0 = starts[np.arange(ncores) * bpc]
    core_r1 = ends[np.arange(ncores) * bpc + bpc - 1]
    rows = core_r1 - core_r0
    Rpad = int(1024 * math.ceil(int(rows.max()) / 1024))
    NJ = Rpad // 1024

    # constants: [wm 5*53 | wm_tail 53 | iota 1]
    wmb = np.zeros((128, NKF * REL + REL + 1), np.float32)
    for k in range(NKF):
        wmb[:, REL * k:REL * (k + 1)] = relmat[:, KCH * k:KCH * (k + 1)].T
    wmb[0:DTAIL, NKF * REL:NKF * REL + REL] = relmat[:, NKF * KCH:].T
    wmb[64:64 + DTAIL, NKF * REL:NKF * REL + REL] = relmat[:, NKF * KCH:].T
    iota = np.zeros(128, np.float32)
    iota[:REL] = np.arange(REL)
    iota[64:64 + REL] = np.arange(REL)
    wmb[:, NKF * REL + REL] = iota
    wmb = wmb.astype(BF16)

    in_maps, metas = [], []
    for c in range(ncores):
        r0, r1 = int(core_r0[c]), int(core_r1[c])
        rc = r1 - r0
        Xc = np.zeros((Rpad, DIM), np.float32)
        Xc[:rc] = repre[r0:r1]
        M = Xc.reshape(NJ, 2, 512, DIM).transpose(3, 0, 1, 2)  # [690,NJ,2,512]
        main = (M[:NKF * KCH].reshape(NKF, KCH, NJ, 2, 512)
                .transpose(1, 2, 0, 3, 4).reshape(KCH, NJ, NKF * 1024))
        tail = np.zeros((KCH, NJ, 512), np.float32)
        tail[0:DTAIL] = M[NKF * KCH:, :, 0, :]
        tail[64:64 + DTAIL] = M[NKF * KCH:, :, 1, :]
        xtb = np.ascontiguousarray(
            np.concatenate([main, tail], axis=2).reshape(128, NJ * BCOL)
        ).astype(BF16)

        lab = np.zeros((1, Rpad), np.float32)
        lab[0, :rc] = labels_np[r0:r1]
        labb = lab.astype(BF16)

        in_maps.append({"xtb": xtb, "wmb": wmb, "labb": labb})
        metas.append((r0, starts[c * bpc:(c + 1) * bpc] - r0,
                      ends[c * bpc:(c + 1) * bpc] - r0, rc))
    return in_maps, metas, bias_np, Rpad, bpc


def _finish(results, metas, bias_np, bpc, Rpad):
    NJ = Rpad // 1024
    out = np.empty((len(results) * bpc, REL), np.float32)
    for c, res in enumerate(results):
        arr = np.asarray(res["outstage"]).astype(np.float32)
        A = arr.reshape(128, NJ, 512)
        pte = np.empty((NJ, 2, 512, AUG), np.float32)   # [block, half, i, r]
        pte[:, 0] = A[0:AUG].transpose(1, 2, 0)
        pte[:, 1] = A[64:64 + AUG].transpose(1, 2, 0)
        pte = pte.reshape(Rpad, AUG)
        _, ls, le, rc = metas[c]
        cs = np.empty((Rpad + 1, AUG), np.float64)
        cs[0] = 0.0
        np.cumsum(pte, axis=0, dtype=np.float64, out=cs[1:])
        sums = cs[le] - cs[ls]
        out[c * bpc:(c + 1) * bpc] = (
            sums[:, :REL] / sums[:, REL:AUG]).astype(np.float32)
    out += bias_np[None, :]
    return out


def kernel(repre, relation_mat, bias, scope, labels):
    global LAST_RESULTS
    from concourse.bass_utils import run_bass_kernel_spmd

    in_maps, metas, bias_np, Rpad, bpc = _prep(
        repre, relation_mat, bias, scope, labels, NCORES)
    if Rpad not in _PROGRAM_CACHE:
        _PROGRAM_CACHE[Rpad] = _build_program(Rpad)
    nc = _PROGRAM_CACHE[Rpad]
    res = run_bass_kernel_spmd(nc, in_maps, core_ids=list(range(NCORES)),
                               trace=bool(os.environ.get("BASS_TRACE")))
    LAST_RESULTS = res
    return _finish(res.results, metas, bias_np, bpc, Rpad)


# revision 10
# speedup vs baseline: 1.6895x; 1.1311x over previous
"""Trainium2 Bass kernel for nn_AttentionSelector (segment softmax attention).

Math shortcut: logits = segment_sum(w * repre) @ relation_mat.T + bias is
linear in repre, so with P = repre @ relation_mat.T ([N,53]) the whole
computation lives in 53-dim space:
    x_i   = P[i, labels[i]]          (rel logit per instance)
    e_i   = exp(x_i)                 (logits are ~N(0, 0.026^2): no max needed)
    out_b = (sum_{i in b} e_i P[i,:]) / (sum_{i in b} e_i) + bias

Device pipeline (bags sharded at the bag boundary nearest each octile of
rows, so every core streams ~25002 rows padded to 25088):
  The X^T stream is the HBM roofline; everything else hides under it.
  Layout per 1024-row block: 5 full 128-d chunks ([128,1024] each) plus
  the 50-dim tail dual-packed (half 0 at partitions 0-49, half 1 at
  64-113) -> 5632 bf16 cols = 11264 B/partition, 98% real data (no
  690->768 zero padding, no one-hot streams). A trailing 512-row block
  carries only half 0 (3072 cols).
  Per block-pair p (even 512-half at partition base 0, odd at 64; the
  two halves' matmuls are issued interleaved so the PE runs them
  concurrently in disjoint column groups via tile_position cols 0/64):
    A:  5 accumulating matmuls + 1 tail matmul (K=50 at base 64h)
        -> P^T in PSUM [53,512]; ACT copies to bf16 pt tile whose
        rows 53-63/117-127 are a permanent ones-block.
    X:  K=1 matmul broadcasts labels row across 53 partitions; one fused
        DVE scalar_tensor_tensor computes junk = (lb == iota) * P^T;
        ones-matmul contracts partitions -> x broadcast to 64 rows in
        PSUM; ACT exp -> e rows; DVE multiplies [P^T; 1] by e writing
        [e P | e] straight into the [128, 512*NP] output staging tile.
  Input streams as up-to-5.8 MB multi-block DMAs (sync engine queue),
  tapering to single blocks at the end to shrink the pipeline drain;
  output flushes every 2 pairs on the scalar engine queue. Host rebuilds
  [N, 54], segment-sums contiguous bags via f64 cumsum-diff, divides,
  adds bias.
"""
import math
import os
import sys

for _p in ("/opt/trn_rl_repo", "/opt/trn_rl_repo/concourse", "/opt/pypackages"):
    if _p not in sys.path:
        sys.path.insert(0, _p)

import numpy as np
import ml_dtypes

BF16 = ml_dtypes.bfloat16

N_TOTAL = 200000
NUM_BAGS = 25000
DIM = 690
KCH = 128
NKF = 5            # full 128-d chunks
DTAIL = DIM - NKF * KCH          # 50
REL = 53
AUG = REL + 1      # 53 P-columns + e column
AUGW = 64          # widened row count so outst rows 54-63/118-127 are
                   # written too (host ignores them; keeps DMA full-width)
BCOL = NKF * 1024 + 512          # 5632 bf16 cols per 1024-row block
SBCOL = NKF * 512 + 512          # 3072 cols for the trailing 512-row block
NCORES = 8
GROUP_BLOCKS = 4                 # steady-state blocks per input DMA
FLUSH_PAIRS = 2                  # output flush granularity (pairs)

LAST_RESULTS = None
_PROGRAM_CACHE = {}


def _in_groups(nfull):
    """Input-DMA group sizes over the full blocks: lead with 1 so compute
    starts early, steady 4, taper to 2,1 to shrink the end drain."""
    groups = [1]
    left = nfull - 1
    while left > 6:
        groups.append(4)
        left -= 4
    while left > 0:
        g = min(2, left)
        groups.append(g)
        left -= g
    return groups


def _build_program(Rpad):
    from concourse import bacc, mybir
    import concourse.tile as tile

    f32 = mybir.dt.float32
    bf16 = mybir.dt.bfloat16
    Alu = mybir.AluOpType
    Act = mybir.ActivationFunctionType
    NJF = Rpad // 1024               # full blocks
    SHORT = (Rpad % 1024) == 512     # trailing 512-row half-block
    NP = NJF + (1 if SHORT else 0)   # pairs
    TOTC = NJF * BCOL + (SBCOL if SHORT else 0)
    IN_GROUPS = _in_groups(NJF)

    nc = bacc.Bacc("TRN2", target_bir_lowering=False, debug=False,
                   enable_asserts=False)

    with tile.TileContext(nc) as tc:
        with tc.tile_pool(name="dram", bufs=1, space="DRAM") as dram, \
             tc.tile_pool(name="consts", bufs=1) as consts, \
             tc.tile_pool(name="xt", bufs=2) as xtp, \
             tc.tile_pool(name="junk", bufs=3) as junkp, \
             tc.tile_pool(name="erow", bufs=3) as erp, \
             tc.tile_pool(name="big", bufs=1) as bigp, \
             tc.tile_pool(name="pt_ps", bufs=2, space="PSUM") as ptps, \
             tc.tile_pool(name="lb_ps", bufs=2, space="PSUM") as lbps, \
             tc.tile_pool(name="x_ps", bufs=2, space="PSUM") as xps:

            xt_d = dram.tile([128, TOTC], bf16, kind="ExternalInput",
                             name="xtb", uniquify=False)
            wm_d = dram.tile([128, NKF * REL + REL + 1], bf16,
                             kind="ExternalInput", name="wmb",
                             uniquify=False)
            lab_d = dram.tile([1, NP * 1024], bf16, kind="ExternalInput",
                              name="labb", uniquify=False)
            out_d = dram.tile([128, NP * 512], bf16, kind="ExternalOutput",
                              name="outstage", uniquify=False)

            # block -> (col offset, per-chunk stride cols, is_short)
            blocks = [(j * BCOL, 1024, False) for j in range(NJF)]
            if SHORT:
                blocks.append((NJF * BCOL, 512, True))
            group_bounds = []
            group_of_block = {}
            g0 = 0
            for g in IN_GROUPS:
                group_bounds.append((g0, g0 + g))
                for j in range(g0, g0 + g):
                    group_of_block[j] = len(group_bounds) - 1
                g0 += g
            if SHORT:
                group_bounds.append((NJF, NJF + 1))
                group_of_block[NJF] = len(group_bounds) - 1

            # the first input group DMA leads every queue: emit it first
            xt_tiles = {}

            def load_group(gi):
                glo, ghi = group_bounds[gi]
                clo = blocks[glo][0]
                chi = blocks[ghi - 1][0] + \
                    (SBCOL if blocks[ghi - 1][2] else BCOL)
                xt = xtp.tile([128, GROUP_BLOCKS * BCOL], bf16,
                              name="xt", tag="xt")
                nc.sync.dma_start(xt[:, :chi - clo], xt_d[:, clo:chi])
                for jj in range(glo, ghi):
                    xt_tiles[jj] = (xt, blocks[jj][0] - clo)

            load_group(0)

            # constants: [wm 5*53 | wm_tail 53 | iota 1]
            # (scalar queue, so xt groups own the sync queue)
            wm_sb = consts.tile([128, NKF * REL + REL + 1], bf16,
                                name="wm_sb", tag="wm_sb")
            nc.scalar.dma_start(wm_sb[:], wm_d[:])
            lab_sb = consts.tile([1, NP * 1024], bf16, name="lab_sb",
                                 tag="lab_sb")
            nc.scalar.dma_start(lab_sb[:], lab_d[:])
            WT = NKF * REL                 # wm_tail col offset
            IOTA = WT + REL                # iota col offset

            onesb = consts.tile([128, AUGW], bf16, name="onesb", tag="onesb")
            nc.vector.memset(onesb[:], 1.0)
            ones1 = consts.tile([1, REL], bf16, name="ones1", tag="ones1")
            nc.vector.memset(ones1[:], 1.0)
            # P^T staging tiles with permanent ones-rows
            pt_sbs = []
            for i in range(4):
                t_ = consts.tile([128, 512], bf16, name=f"pt_sb{i}",
                                 tag=f"pt_sb{i}")
                nc.vector.memset(t_[:], 1.0)
                pt_sbs.append(t_)

            outst = bigp.tile([128, NP * 512], bf16, name="outst",
                              tag="outst")
            if SHORT:
                # odd-half partitions of the trailing half-pair are never
                # computed; zero them so the flush DMA reads defined data
                nc.vector.memset(outst[64:128, (NP - 1) * 512:], 0.0)

            pt_ps_t = {}
            out_state = {"done": 0}

            def maybe_flush(pdone, final=False):
                hi = (pdone + 1) * 512
                if final:
                    hi = NP * 512
                if hi - out_state["done"] >= FLUSH_PAIRS * 512 or \
                        (final and hi > out_state["done"]):
                    lo = out_state["done"]
                    nc.scalar.dma_start(out_d[:, lo:hi], outst[:, lo:hi])
                    out_state["done"] = hi

            for p in range(NP + 1):
                u = p - 1
                uhalves = None
                if 0 <= u < NP:
                    uhalves = (0,) if (SHORT and u == NP - 1) else (0, 1)

                # ---- stage X part 1 for pair p-1 ----
                if uhalves:
                    lb_ps = lbps.tile([128, 512], f32, space="PSUM",
                                      name="lb_ps", tag="lb_ps")
                    for hu in uhalves:
                        bu = 64 * hu
                        nc.tensor.matmul(
                            lb_ps[bu:bu + REL, :], ones1[:, :],
                            lab_sb[:, 1024 * u + 512 * hu:
                                   1024 * u + 512 * (hu + 1)],
                            start=True, stop=True)
                    pt_sb = pt_sbs[u % 4]
                    junk = junkp.tile([128, 512], bf16, name="junk",
                                      tag="junk")
                    for hu in uhalves:
                        bu = 64 * hu
                        nc.scalar.activation(
                            pt_sb[bu:bu + REL, :],
                            pt_ps_t[u][bu:bu + REL, :], Act.Copy)
                        # junk = (lb == iota) * P^T  in one DVE op
                        nc.vector.scalar_tensor_tensor(
                            out=junk[bu:bu + REL, :],
                            in0=lb_ps[bu:bu + REL, :],
                            scalar=wm_sb[bu:bu + REL, IOTA:IOTA + 1],
                            in1=pt_sb[bu:bu + REL, :],
                            op0=Alu.is_equal, op1=Alu.mult)

                # ---- stage A: interleaved matmuls for pair p ----
                if p < NP:
                    gi = group_of_block[p]
                    if p == group_bounds[gi][0] and p > 0:
                        load_group(gi)
                    xt, off = xt_tiles[p]
                    _, cstride, is_short = blocks[p]
                    halves = (0,) if is_short else (0, 1)
                    pt_ps = ptps.tile([128, 512], f32, space="PSUM",
                                      name="pt_ps", tag="pt_ps")
                    for k in range(NKF):
                        for h in halves:
                            b = 64 * h
                            nc.tensor.matmul(
                                pt_ps[b:b + REL, :],
                                wm_sb[:, REL * k:REL * (k + 1)],
                                xt[:, off + cstride * k + 512 * h:
                                   off + cstride * k + 512 * (h + 1)],
                                start=(k == 0), stop=False,
                                skip_group_check=True)
                    for h in halves:
                        b = 64 * h
                        nc.tensor.matmul(
                            pt_ps[b:b + REL, :],
                            wm_sb[b:b + DTAIL, WT:WT + REL],
                            xt[b:b + DTAIL, off + NKF * cstride:
                               off + NKF * cstride + 512],
                            start=False, stop=True,
                            skip_group_check=True)
                    pt_ps_t[p] = pt_ps

                # ---- stage X part 2 for pair p-1 ----
                if uhalves:
                    xT_ps = xps.tile([128, 512], f32, space="PSUM",
                                     name="xT", tag="xT")
                    for hu in uhalves:
                        bu = 64 * hu
                        nc.tensor.matmul(
                            xT_ps[bu:bu + AUGW, :], onesb[bu:bu + REL, :],
                            junk[bu:bu + REL, :], start=True, stop=True)
                    e_bc = erp.tile([128, 512], bf16, name="erow",
                                    tag="erow")
                    pt_sb = pt_sbs[u % 4]
                    for hu in uhalves:
                        bu = 64 * hu
                        nc.scalar.activation(
                            e_bc[bu:bu + AUGW, :], xT_ps[bu:bu + AUGW, :],
                            Act.Exp)
                        nc.vector.tensor_tensor(
                            out=outst[bu:bu + AUGW,
                                      512 * u:512 * (u + 1)],
                            in0=pt_sb[bu:bu + AUGW, :],
                            in1=e_bc[bu:bu + AUGW, :], op=Alu.mult)
                    pt_ps_t.pop(u)
                    maybe_flush(u)
            maybe_flush(NP - 1, final=True)

    nc.compile()
    return nc


def _core_cuts(starts, ncores, n_total):
    """Bag-boundary cuts closest to equal row octiles."""
    cuts = [0]
    nbags = len(starts)
    for c in range(1, ncores):
        target = c * n_total // ncores
        i = int(np.searchsorted(starts, target))
        if i > 0 and abs(int(starts[i - 1]) - target) < \
                abs(int(starts[i]) - target):
            i -= 1
        cuts.append(i)
    cuts.append(nbags)
    return cuts


def _prep(repre, relation_mat, bias, scope, labels, ncores):
    repre = np.asarray(repre, dtype=np.float32)
    relmat = np.asarray(relation_mat, dtype=np.float32)
    bias_np = np.asarray(bias, dtype=np.float32)
    scope = np.asarray(scope).astype(np.int64)
    labels_np = np.asarray(labels).astype(np.int64)
    n, d = repre.shape
    assert d == DIM
    starts, ends = scope[:, 0], scope[:, 1]
    cuts = _core_cuts(starts, ncores, n)
    core_r0 = np.array([starts[cuts[c]] for c in range(ncores)])
    core_r1 = np.array([ends[cuts[c + 1] - 1] for c in range(ncores)])
    rows = core_r1 - core_r0
    Rpad = int(512 * math.ceil(int(rows.max()) / 512))
    NJF = Rpad // 1024
    SHORT = (Rpad % 1024) == 512
    NP = NJF + (1 if SHORT else 0)

    # constants: [wm 5*53 | wm_tail 53 | iota 1]
    wmb = np.zeros((128, NKF * REL + REL + 1), np.float32)
    for k in range(NKF):
        wmb[:, REL * k:REL * (k + 1)] = relmat[:, KCH * k:KCH * (k + 1)].T
    wmb[0:DTAIL, NKF * REL:NKF * REL + REL] = relmat[:, NKF * KCH:].T
    wmb[64:64 + DTAIL, NKF * REL:NKF * REL + REL] = relmat[:, NKF * KCH:].T
    iota = np.zeros(128, np.float32)
    iota[:REL] = np.arange(REL)
    iota[64:64 + REL] = np.arange(REL)
    wmb[:, NKF * REL + REL] = iota
    wmb = wmb.astype(BF16)

    in_maps, metas = [], []
    for c in range(ncores):
        r0, r1 = int(core_r0[c]), int(core_r1[c])
        rc = r1 - r0
        Xc = np.zeros((NP * 1024, DIM), np.float32)
        Xc[:rc] = repre[r0:r1]
        M = Xc[:NJF * 1024].reshape(NJF, 2, 512, DIM) \
            .transpose(3, 0, 1, 2)               # [690, NJF, 2, 512]
        main = (M[:NKF * KCH].reshape(NKF, KCH, NJF, 2, 512)
                .transpose(1, 2, 0, 3, 4).reshape(KCH, NJF, NKF * 1024))
        tail = np.zeros((KCH, NJF, 512), np.float32)
        tail[0:DTAIL] = M[NKF * KCH:, :, 0, :]
        tail[64:64 + DTAIL] = M[NKF * KCH:, :, 1, :]
        xtb_full = np.concatenate([main, tail], axis=2) \
            .reshape(128, NJF * BCOL)
        parts = [xtb_full]
        if SHORT:
            Ms = Xc[NJF * 1024:NJF * 1024 + 512].T       # [690, 512]
            smain = Ms[:NKF * KCH].reshape(NKF, KCH, 512) \
                .transpose(1, 0, 2).reshape(KCH, NKF * 512)
            stail = np.zeros((KCH, 512), np.float32)
            stail[0:DTAIL] = Ms[NKF * KCH:]
            parts.append(np.concatenate([smain, stail], axis=1))
        xtb = np.ascontiguousarray(
            np.concatenate(parts, axis=1)).astype(BF16)

        lab = np.zeros((1, NP * 1024), np.float32)
        lab[0, :rc] = labels_np[r0:r1]
        labb = lab.astype(BF16)

        in_maps.append({"xtb": xtb, "wmb": wmb, "labb": labb})
        metas.append((starts[cuts[c]:cuts[c + 1]] - r0,
                      ends[cuts[c]:cuts[c + 1]] - r0, rc))
    return in_maps, metas, bias_np, Rpad


def _finish(results, metas, bias_np, Rpad):
    NJF = Rpad // 1024
    SHORT = (Rpad % 1024) == 512
    NP = NJF + (1 if SHORT else 0)
    outs = []
    for c, res in enumerate(results):
        arr = np.asarray(res["outstage"]).astype(np.float32)
        A = arr.reshape(128, NP, 512)
        pte = np.empty((NP, 2, 512, AUG), np.float32)   # [pair, half, i, r]
        pte[:, 0] = A[0:AUG].transpose(1, 2, 0)
        pte[:, 1] = A[64:64 + AUG].transpose(1, 2, 0)
        pte = pte.reshape(NP * 1024, AUG)
        ls, le, rc = metas[c]
        cs = np.empty((NP * 1024 + 1, AUG), np.float64)
        cs[0] = 0.0
        np.cumsum(pte, axis=0, dtype=np.float64, out=cs[1:])
        sums = cs[le] - cs[ls]
        outs.append((sums[:, :REL] / sums[:, REL:AUG]).astype(np.float32))
    out = np.concatenate(outs, axis=0)
    out += bias_np[None, :]
    return out


def kernel(repre, relation_mat, bias, scope, labels):
    global LAST_RESULTS
    from concourse.bass_utils import run_bass_kernel_spmd

    in_maps, metas, bias_np, Rpad = _prep(
        repre, relation_mat, bias, scope, labels, NCORES)
    if Rpad not in _PROGRAM_CACHE:
        _PROGRAM_CACHE[Rpad] = _build_program(Rpad)
    nc = _PROGRAM_CACHE[Rpad]
    res = run_bass_kernel_spmd(nc, in_maps, core_ids=list(range(NCORES)),
                               trace=bool(os.environ.get("BASS_TRACE")))
    LAST_RESULTS = res
    return _finish(res.results, metas, bias_np, Rpad)
